# revision 7
# baseline (speedup 1.0000x reference)
"""Trainium2 Bass kernel for Cross-MultiAttention, v7.

v2 (kernel2) + K-padded QK matmuls: the runtime charges ~2x for matmuls
with contraction < 128, so Q/K are stored per-head in 128-partition
buffers with complementary zero halves (even head h=2m in partitions
0:64, odd h=2m+1 in 64:128, the other half zeroed once at setup). Each
QK matmul then contracts a full 128 partitions (64 real + 64 zeros) at
the cheap rate. Projection evictions split into two partition-aligned
halves writing the padded layout directly (no partition shift needed).

proj_out packs head PAIRS into full-128 contractions (16 matmuls, not
32): ocat slot m holds head 2m on partitions 0:64 and head 2m+1 on
64:128; the odd head's normalized output is produced on 0:64 and moved
up by a partition-shifted SBUF->SBUF DMA (the only engine that can
cross partitions).

Also: zT stored fp8 (exact for 0/1) and the output staging buffer
chunked to fit SBUF alongside the padded Q/K.

See kernel2.py for the sharding, host-side algebraic folds, and the
rest of the dataflow (all unchanged).
"""

import time as _time

import numpy as np
import ml_dtypes

import concourse.bass as bass
import concourse.tile as tile
import concourse.mybir as mybir
from concourse.bacc import Bacc
from concourse.bass_utils import run_bass_kernel_spmd

BF16 = mybir.dt.bfloat16
F32 = mybir.dt.float32
F8 = mybir.dt.float8e4
AF = mybir.ActivationFunctionType

B, T, S, C, E, H = 4, 2048, 2048, 256, 512, 8
D = E // H
SCALE = float(E) ** -0.5
NCORES = 8
HL = H // 2
EL = HL * D
NJ = S // 128
KE = E // 128
KC = C // 128
ML = EL // 128
MC = C // 128
HW = HL * 65

_NC_CACHE = {}


def _build_nc(repeat=1):
    nc = Bacc("TRN2", target_bir_lowering=False, debug=False)

    xT = nc.dram_tensor("xT", [128, KC, T], BF16, kind="ExternalInput")
    ctxT = nc.dram_tensor("ctxT", [128, KE, S], BF16, kind="ExternalInput")
    zT = nc.dram_tensor("zT", [128, NJ, T], F8, kind="ExternalInput")
    wqT = nc.dram_tensor("wqT", [128, KC, EL], BF16, kind="ExternalInput")
    wkT = nc.dram_tensor("wkT", [128, KE, EL], BF16, kind="ExternalInput")
    wvT = nc.dram_tensor("wvT", [128, KE, EL], BF16, kind="ExternalInput")
    w_outT = nc.dram_tensor("w_outT", [128, ML, C], BF16, kind="ExternalInput")
    bq = nc.dram_tensor("bq", [128, ML], F32, kind="ExternalInput")
    bk = nc.dram_tensor("bk", [128, ML], F32, kind="ExternalInput")
    bo = nc.dram_tensor("bo", [128, MC], F32, kind="ExternalInput")
    outT = nc.dram_tensor("outT", [128, MC, T], F32, kind="ExternalOutput")

    with tile.TileContext(nc) as tc:
        with tc.tile_pool(name="const", bufs=1) as cp, \
             tc.tile_pool(name="acts", bufs=1) as ap, \
             tc.tile_pool(name="pe", bufs=1) as pe_p, \
             tc.tile_pool(name="pm", bufs=1) as pm_p, \
             tc.tile_pool(name="fo", bufs=2) as fo_p, \
             tc.tile_pool(name="nrm", bufs=1) as nrm, \
             tc.tile_pool(name="big", bufs=1, space="PSUM") as ps_big, \
             tc.tile_pool(name="av", bufs=1, space="PSUM") as ps_av:

            # ---- persistent loads ----
            wqT_sb = cp.tile([128, KC, EL], BF16, tag="wqT")
            nc.sync.dma_start(out=wqT_sb, in_=wqT[:, :, :])
            xT_sb = cp.tile([128, KC, T], BF16, tag="xT")
            nc.sync.dma_start(out=xT_sb, in_=xT[:, :, :])
            bq_sb = cp.tile([128, ML], F32, tag="bq")
            nc.sync.dma_start(out=bq_sb, in_=bq[:, :])
            bk_sb = cp.tile([128, ML], F32, tag="bk")
            nc.sync.dma_start(out=bk_sb, in_=bk[:, :])
            bo_sb = cp.tile([128, MC], F32, tag="bo")
            nc.sync.dma_start(out=bo_sb, in_=bo[:, :])
            wkT_sb = cp.tile([128, KE, EL], BF16, tag="wkT")
            nc.sync.dma_start(out=wkT_sb, in_=wkT[:, :, :])
            ctxT_sb = cp.tile([128, KE, S], BF16, tag="ctxT")
            nc.sync.dma_start(out=ctxT_sb, in_=ctxT[:, :, :])
            wvT_sb = cp.tile([128, KE, EL], BF16, tag="wvT")
            nc.sync.dma_start(out=wvT_sb, in_=wvT[:, :, :])
            w_outT_sb = cp.tile([128, ML, C], BF16, tag="w_outT")
            nc.sync.dma_start(out=w_outT_sb, in_=w_outT[:, :, :])
            zT_sb = cp.tile([128, NJ, T], F8, tag="zT")
            for j in range(NJ):
                nc.sync.dma_start(out=zT_sb[:, j, :], in_=zT[:, j, :])

            # ---- persistent activations ----
            # K padded per head: head 2m in partitions 0:64 of slot 2m,
            # head 2m+1 in partitions 64:128 of slot 2m+1; other halves
            # zeroed once so each QK matmul contracts a full 128 rows
            # (the zeros null the other head's Q rows in the moving op).
            QT_sb = ap.tile([128, ML, T], BF16, tag="QT")
            KT_sb = ap.tile([128, HL, S], BF16, tag="KT")
            nc.vector.memset(KT_sb[64:128, 0::2, :], 0.0)
            nc.vector.memset(KT_sb[0:64, 1::2, :], 0.0)
            V_sb = ap.tile([128, NJ, HW], BF16, tag="V")
            nc.vector.memset(
                V_sb[:, :, :].rearrange("p j (h w) -> p j h w", w=65)
                [:, :, :, 64:65], 1.0)
            ocat_sb = ap.tile([128, ML, T], BF16, tag="ocat")

            for _rep in range(repeat):
                # ---- Q projection (fused proj_in), padded eviction ----
                for m in range(ML):
                    p = ps_big.tile([128, 2048], F32, tag="big",
                                    name=f"q_{m}_{_rep}")
                    for t in range(4):
                        for k in range(KC):
                            nc.tensor.matmul(
                                p[:, t * 512:(t + 1) * 512],
                                wqT_sb[:, k, m * 128:(m + 1) * 128],
                                xT_sb[:, k, t * 512:(t + 1) * 512],
                                start=(k == 0), stop=(k == KC - 1))
                    nc.scalar.activation(out=QT_sb[:, m, :], in_=p[:, :],
                                         func=AF.Identity,
                                         bias=bq_sb[:, m:m + 1], scale=1.0)

                # ---- K projection, padded eviction ----
                for m in range(ML):
                    p = ps_big.tile([128, 2048], F32, tag="big",
                                    name=f"k_{m}_{_rep}")
                    for t in range(4):
                        for k in range(KE):
                            nc.tensor.matmul(
                                p[:, t * 512:(t + 1) * 512],
                                wkT_sb[:, k, m * 128:(m + 1) * 128],
                                ctxT_sb[:, k, t * 512:(t + 1) * 512],
                                start=(k == 0), stop=(k == KE - 1))
                    nc.scalar.activation(out=KT_sb[0:64, 2 * m, :],
                                         in_=p[0:64, :], func=AF.Identity,
                                         bias=bk_sb[0:64, m:m + 1], scale=1.0)
                    nc.scalar.activation(out=KT_sb[64:128, 2 * m + 1, :],
                                         in_=p[64:128, :], func=AF.Identity,
                                         bias=bk_sb[64:128, m:m + 1], scale=1.0)

                # ---- V projection: [s, e], 8 s-chunks per PSUM tile ----
                for sg in range(2):
                    p = ps_big.tile([128, 2048], F32, tag="big",
                                    name=f"v_{sg}_{_rep}")
                    for st8 in range(8):
                        st = sg * 8 + st8
                        for k in range(KE):
                            nc.tensor.matmul(
                                p[:, st8 * EL:(st8 + 1) * EL],
                                ctxT_sb[:, k, st * 128:(st + 1) * 128],
                                wvT_sb[:, k, :],
                                start=(k == 0), stop=(k == KE - 1))
                    dst = V_sb[:, sg * 8:(sg + 1) * 8, :].rearrange(
                        "p s (h w) -> p s h w", w=65)[:, :, :, 0:64]
                    src = p[:, :].rearrange("p (s h w) -> p s h w", s=8, h=HL)
                    nc.scalar.activation(out=dst, in_=src, func=AF.Identity,
                                         scale=1.0)

                # ---- attention (K-padded QK at full 128 contraction) ----
                for h in range(HL):
                    et = h // 2
                    oav = ps_av.tile([65, 2048], F32, tag="av",
                                     name=f"oav_{h}_{_rep}")
                    for jq in range(NJ // 4):
                        pe_t = pe_p.tile([128, 8192], BF16, tag="pexp")
                        for jh in range(4):
                            j = jq * 4 + jh
                            pqk = ps_big.tile([128, 2048], F32, tag="big",
                                              name=f"qk_{h}_{j}_{_rep}")
                            for t in range(4):
                                nc.tensor.matmul(
                                    pqk[:, t * 512:(t + 1) * 512],
                                    KT_sb[:, h, j * 128:(j + 1) * 128],
                                    QT_sb[:, et, t * 512:(t + 1) * 512],
                                    start=True, stop=True)
                            nc.scalar.activation(
                                out=pe_t[:, jh * 2048:(jh + 1) * 2048],
                                in_=pqk[:, :], func=AF.Exp, scale=SCALE)
                        pm_t = pm_p.tile([128, 8192], BF16, tag="pmask")
                        nc.vector.tensor_mul(
                            pm_t[:, :], pe_t[:, :],
                            zT_sb[:, jq * 4:jq * 4 + 4, :].rearrange(
                                "p a b -> p (a b)"))
                        for jh in range(4):
                            j = jq * 4 + jh
                            for t in range(4):
                                nc.tensor.matmul(
                                    oav[:, t * 512:(t + 1) * 512],
                                    V_sb[:, j, h * 65:(h + 1) * 65],
                                    pm_t[:, jh * 2048 + t * 512:
                                         jh * 2048 + (t + 1) * 512],
                                    start=(j == 0), stop=(j == NJ - 1))
                    rec = nrm.tile([1, 2048], F32, tag="rec")
                    nc.vector.reciprocal(rec[0:1, :], oav[64:65, :])
                    rb = nrm.tile([64, 2048], F32, tag="rb")
                    nc.gpsimd.partition_broadcast(rb[:, :], rec[0:1, :])
                    if h % 2 == 0:
                        nc.vector.tensor_mul(ocat_sb[0:64, h // 2, :],
                                             oav[0:64, :], rb[:, :])
                    else:
                        otmp = nrm.tile([64, 2048], BF16, tag="otmp")
                        nc.vector.tensor_mul(otmp[:, :], oav[0:64, :], rb[:, :])
                        nc.sync.dma_start(out=ocat_sb[64:128, h // 2, :],
                                          in_=otmp[0:64, :])

                # ---- proj_out (partial; host sums the pair) ----
                for m in range(MC):
                    pf = ps_big.tile([128, 2048], F32, tag="big",
                                     name=f"pf_{m}_{_rep}")
                    for t in range(4):
                        for mp in range(ML):
                            nc.tensor.matmul(
                                pf[:, t * 512:(t + 1) * 512],
                                w_outT_sb[:, mp, m * 128:(m + 1) * 128],
                                ocat_sb[:, mp, t * 512:(t + 1) * 512],
                                start=(mp == 0), stop=(mp == ML - 1))
                    fo = fo_p.tile([128, 2048], F32, tag="fo",
                                   name=f"fo_{m}_{_rep}")
                    nc.scalar.activation(out=fo[:, :], in_=pf[:, :],
                                         func=AF.Identity,
                                         bias=bo_sb[:, m:m + 1], scale=1.0)
                    nc.sync.dma_start(out=outT[:, m, :], in_=fo[:, :])

    nc.finalize()
    return nc


def get_nc(repeat=1):
    if repeat not in _NC_CACHE:
        _NC_CACHE[repeat] = _build_nc(repeat)
    return _NC_CACHE[repeat]


def _pack(a, p=128):
    k = a.shape[0] // p
    return np.ascontiguousarray(
        a.reshape(k, p, *a.shape[1:]).transpose(1, 0, *range(2, a.ndim + 1)))


def build_in_maps(x, context, pad_mask, w_in, b_in, wq, bq, wk, bk, wv, bv,
                  w_out, b_out):
    bf = ml_dtypes.bfloat16
    f8 = ml_dtypes.float8_e4m3
    f32 = np.float32
    x = np.asarray(x, f32); context = np.asarray(context, f32)
    w_in = np.asarray(w_in, f32); b_in = np.asarray(b_in, f32)
    wq = np.asarray(wq, f32); bq = np.asarray(bq, f32)
    wk = np.asarray(wk, f32); bk = np.asarray(bk, f32)
    wv = np.asarray(wv, f32); bv = np.asarray(bv, f32)
    w_out = np.asarray(w_out, f32); b_out = np.asarray(b_out, f32)

    wq_eff = wq @ w_in                        # [E, C]
    bq_eff = wq @ b_in + bq
    bo_eff = b_out + w_out @ bv
    z = (~np.asarray(pad_mask)).astype(f8)    # [B, T, S]

    wq_g, wk_g, wv_g, wo_g, bq_g, bk_g = [], [], [], [], [], []
    for hg in range(2):
        sl = slice(hg * EL, (hg + 1) * EL)
        wq_g.append(_pack(np.ascontiguousarray(wq_eff.T[:, sl]).astype(bf)))
        wk_g.append(_pack(np.ascontiguousarray(wk.T[:, sl]).astype(bf)))
        wv_g.append(_pack(np.ascontiguousarray(wv.T[:, sl]).astype(bf)))
        woh = w_out.T[sl, :].reshape(HL, 64, C)      # [head, 64, C]
        wo = np.concatenate(
            [np.concatenate([woh[2 * mp], woh[2 * mp + 1]], axis=0)[:, None, :]
             for mp in range(ML)], axis=1)            # [128, ML, C]
        wo_g.append(np.ascontiguousarray(wo).astype(bf))
        bq_g.append(np.ascontiguousarray(
            bq_eff[sl].reshape(ML, 128).T).astype(f32))
        bk_g.append(np.ascontiguousarray(bk[sl].reshape(ML, 128).T).astype(f32))
    bo_p = np.ascontiguousarray(bo_eff.reshape(MC, 128).T).astype(f32)
    bo_zero = np.zeros_like(bo_p)

    in_maps = []
    for c in range(NCORES):
        b, hg = c // 2, c % 2
        in_maps.append({
            "xT": _pack(np.ascontiguousarray(x[b].T).astype(bf)),
            "ctxT": _pack(np.ascontiguousarray(context[b].T).astype(bf)),
            "zT": _pack(np.ascontiguousarray(z[b].T)),
            "wqT": wq_g[hg], "wkT": wk_g[hg], "wvT": wv_g[hg],
            "w_outT": wo_g[hg],
            "bq": bq_g[hg], "bk": bk_g[hg],
            "bo": bo_p if hg == 0 else bo_zero,
        })
    return in_maps


def assemble_output(results):
    out = np.empty((B, T, C), dtype=np.float32)
    for b in range(B):
        acc = results[2 * b]["outT"] + results[2 * b + 1]["outT"]
        ct = acc.transpose(1, 0, 2).reshape(C, T)
        out[b] = ct.T
    return out


def run(in_maps, repeat=1, **kw):
    # One retry shields against transient device faults
    # (NRT_EXEC_UNIT_UNRECOVERABLE observed on this terminal).
    try:
        return run_bass_kernel_spmd(get_nc(repeat), in_maps,
                                    core_ids=list(range(NCORES)), **kw)
    except Exception:
        _time.sleep(10)
        return run_bass_kernel_spmd(get_nc(repeat), in_maps,
                                    core_ids=list(range(NCORES)), **kw)


def kernel(**inputs):
    in_maps = build_in_maps(
        np.asarray(inputs["x"]), np.asarray(inputs["context"]),
        np.asarray(inputs["pad_mask"]), np.asarray(inputs["w_in"]),
        np.asarray(inputs["b_in"]), np.asarray(inputs["wq"]),
        np.asarray(inputs["bq"]), np.asarray(inputs["wk"]),
        np.asarray(inputs["bk"]), np.asarray(inputs["wv"]),
        np.asarray(inputs["bv"]), np.asarray(inputs["w_out"]),
        np.asarray(inputs["b_out"]))
    res = run(in_maps)
    return assemble_output(res.results)
